# revision 34
# baseline (speedup 1.0000x reference)
"""Trainium2 Bass kernel for nn_BayesianClassifier (MC-dropout 1x1-conv classifier).

Math: out = logit( mean_s sigmoid( (W * mask_s * 2) @ f + b ) )

Key restructuring vs the reference:
  - dropout2d masks fold into per-sample weight matrices on the host
    (einsum(f * m, W) == einsum(f, W * m)), so features are read ONCE.
  - Data-parallel over batch: core b handles features[b] ([256, 40000]).
  - Per core: 40 samples x 14 classes = 560 output rows, processed as
    5 groups of 112 live rows ((s_local, k) pairs, 8 samples/group),
    padded to 128 rows so every stationary operand has exactly 128
    columns (enables PE Fast Weight Load).
    Per hw-chunk of 256 columns:
      * 10 matmuls (5 groups x 2 c-halves) fp16 -> PSUM [128, 5, 256] (3 banks)
      * 1 merged sigmoid ACT (per-partition bias = bias[k]) -> SBUF fp16
      * DVE pair-add + add, GpSimd final add: sums the 5 groups -> [128, 256]
      * selector matmul (fp16 0/1 matrix) contracts the 8 samples and
        scatters the 14 classes of chunk i into rows 14i..14i+14 of a
        PSUM accumulator [128, 256] shared by 9 consecutive chunks.
  - Totals are packed into SBUF; the logit epilogue (Ln(t) - Ln(40-t))
    is forced after all sigmoids (explicit dep) so the ACT table set
    switches exactly once.
"""

import numpy as np

B, C, H, W = 8, 256, 200, 200
S, K = 40, 14
HW = H * W  # 40000
GROUPS = 5
SPG = 8  # samples per group
M = SPG * K  # 112 live rows per matmul group
MP = 128  # padded rows (FWL needs 128-column stationary operands)
CHUNK = 304  # 5*304*4B = 6080 <= 3 PSUM banks; fewer chunks -> fewer ACT
# instruction overheads (352 cyc each) on the bottleneck scalar queue
BANK_F32 = 512
SUPER = 9  # chunks per supertile
M2 = SUPER * K  # 126 live totals rows
M2P = 128
NCORES = 8

_CACHE = {}


def _chunk_layout(hw_total):
    """Returns list of supertiles; each is (hw_offset, [chunk_widths]).
    The first supertile is short (128+256 cols) so its feature DMA lands
    fast and the first sigmoid issues as early as possible."""
    sts = []
    off = 0
    first = True
    while off < hw_total:
        if first:
            widths = [128, CHUNK]
            first = False
        else:
            w_st = min(SUPER * CHUNK, hw_total - off)
            widths = []
            rem = w_st
            while rem > 0:
                cw = min(CHUNK, rem)
                widths.append(cw)
                rem -= cw
        sts.append((off, widths))
        off += sum(widths)
    return sts


def _bank_windows(g, cw):
    """Split group g's [g*CHUNK, g*CHUNK+cw) PSUM window at 512-f32 bank
    lines: a single matmul output must stay within one PSUM bank."""
    lo, hi = g * CHUNK, g * CHUNK + cw
    cuts = [lo]
    b = (lo // BANK_F32 + 1) * BANK_F32
    while b < hi:
        cuts.append(b)
        b += BANK_F32
    cuts.append(hi)
    return [(a - lo, b2 - a) for a, b2 in zip(cuts, cuts[1:])]


def _build_program(hw_total):
    import concourse.bass as bass
    import concourse.bacc as bacc
    import concourse.tile as tile
    import concourse.mybir as mybir

    dt = mybir.dt
    f16, f32 = dt.float16, dt.float32

    nc = bacc.Bacc("TRN2", target_bir_lowering=False, debug=False)

    fh_d = nc.dram_tensor("fh", [C, hw_total], f16, kind="ExternalInput")
    wall_d = nc.dram_tensor("wall", [C, GROUPS * MP], f16, kind="ExternalInput")
    sel_d = nc.dram_tensor("sel", [MP, SUPER, M2P], f16, kind="ExternalInput")
    bias_d = nc.dram_tensor("biasv", [MP, 1], f32, kind="ExternalInput")

    sts = _chunk_layout(hw_total)
    n_st = len(sts)
    # output stays in the packed (chunk*14+k, st*256+col) totals layout; the
    # host un-permutes. A [K, hw] scatter DMA costs 126 descriptors/supertile
    # (~1.1us queue issue each, ~21us total); the packed layout is one fat
    # contiguous DMA per epilogue segment.
    out_d = nc.dram_tensor("out", [M2P, n_st * CHUNK], f32, kind="ExternalOutput")

    sig_insts = []
    ln_insts = []

    with tile.TileContext(nc) as tc:
        with (
            tc.tile_pool(name="const", bufs=1) as constp,
            tc.tile_pool(name="fpool", bufs=2) as fpool,
            tc.tile_pool(name="sigp", bufs=7) as sigp,
            tc.tile_pool(name="treep", bufs=7) as treep,
            tc.tile_pool(name="totsb", bufs=1) as totsb,
            tc.tile_pool(name="epi", bufs=1) as epi,
            tc.tile_pool(name="psl", bufs=2, space=bass.MemorySpace.PSUM) as psl,
            tc.tile_pool(name="pst", bufs=2, space=bass.MemorySpace.PSUM) as pst,
        ):
            # first supertile's feature tiles get the first sync-queue slots
            # (everything downstream waits on them); consts follow
            off0, widths0 = sts[0]
            w_st0 = sum(widths0)
            f0_first = fpool.tile(
                [128, w_st0], f16, tag="f0", padded_shape=[128, SUPER * CHUNK]
            )
            f1_first = fpool.tile(
                [128, w_st0], f16, tag="f1", padded_shape=[128, SUPER * CHUNK]
            )
            wall0 = constp.tile([128, GROUPS * MP], f16)
            wall1 = constp.tile([128, GROUPS * MP], f16)
            sel_s = constp.tile([MP, SUPER, M2P], f16)
            bias_s = constp.tile([MP, 1], f32)
            czero = constp.tile([M2P, 1], f32)
            cS = constp.tile([M2P, 1], f32)
            warm = constp.tile([128, 512], f16)
            scratch = constp.tile([M2P, 1], f32)
            nc.gpsimd.memset(warm[:], 0.0)
            nc.gpsimd.memset(czero[:], 0.0)
            nc.gpsimd.memset(cS[:], float(S))
            # spread the startup DMAs across 3 DMA-capable queues: each
            # dma_start costs ~600ns of queue issue time, so serializing them
            # on one queue delays the first matmul by ~4us
            nc.sync.dma_start(f0_first[:], fh_d[0:128, off0 : off0 + w_st0])
            nc.gpsimd.dma_start(wall0[:], wall_d[0:128, :])
            nc.scalar.dma_start(f1_first[:], fh_d[128:256, off0 : off0 + w_st0])
            nc.sync.dma_start(wall1[:], wall_d[128:256, :])
            nc.gpsimd.dma_start(bias_s[:], bias_d[:])
            nc.sync.dma_start(sel_s[:], sel_d[:])
            # dummy first-in-program sigmoid with no DMA deps: hoists the
            # sigmoid ACT_TABLE_LOAD to ~7us (it otherwise sits behind the
            # first chunk's matmul-wait sems, costing 1.3us on the critical
            # path)
            nc.scalar.activation(
                scratch[:],
                czero[:],
                mybir.ActivationFunctionType.Sigmoid,
                bias=czero[:],
            )
            # PE warm-up: HAM un-throttles after ~3.4us of sustained matmul
            # activity; without this, chunks 0-2 run at the 1.2GHz cold rate
            # while the feature/weight DMAs land (~5us of ramp)
            wtile = psl.tile([MP, GROUPS, CHUNK], f32, tag="logits")
            for _ in range(12):
                nc.tensor.matmul(
                    wtile[:, 0, :],
                    warm[:, 0:128],
                    warm[:, 0:CHUNK],
                    start=True,
                    stop=True,
                )

            totals_sb = totsb.tile([M2P, n_st * CHUNK], f32)

            # ---------------- main loop ----------------
            # sel matmuls are emitted SEL_LAG chunks late (global queue,
            # crossing supertile boundaries) so the sigmoid->DVE->GpSimd
            # chain latency never stalls the PE FIFO.
            SEL_LAG = 4
            pending_sels = []
            for st, (off, widths) in enumerate(sts):
                w_st = sum(widths)
                if st == 0:
                    f0, f1 = f0_first, f1_first
                else:
                    f0 = fpool.tile(
                        [128, w_st], f16, tag="f0", padded_shape=[128, SUPER * CHUNK]
                    )
                    f1 = fpool.tile(
                        [128, w_st], f16, tag="f1", padded_shape=[128, SUPER * CHUNK]
                    )
                    nc.sync.dma_start(f0[:], fh_d[0:128, off : off + w_st])
                    nc.gpsimd.dma_start(f1[:], fh_d[128:256, off : off + w_st])

                tot_ps = pst.tile([M2P, CHUNK], f32, tag="totals")

                c0 = 0
                for i, cw in enumerate(widths):
                    logits = psl.tile([MP, GROUPS, CHUNK], f32, tag="logits")
                    for g in range(GROUPS):
                        for wo, ww in _bank_windows(g, cw):
                            nc.tensor.matmul(
                                logits[:, g, wo : wo + ww],
                                wall0[:, g * MP : (g + 1) * MP],
                                f0[:, c0 + wo : c0 + wo + ww],
                                start=True,
                                stop=False,
                            )
                            nc.tensor.matmul(
                                logits[:, g, wo : wo + ww],
                                wall1[:, g * MP : (g + 1) * MP],
                                f1[:, c0 + wo : c0 + wo + ww],
                                start=False,
                                stop=True,
                            )
                    while len(pending_sels) >= 2 * SEL_LAG:
                        for _ in range(2):
                            a, k, pack = pending_sels.pop(0)
                            nc.tensor.matmul(*a, **k)
                            if pack is not None:
                                src_ps, pst_idx = pack
                                nc.vector.tensor_copy(
                                    totals_sb[
                                        0:M2, pst_idx * CHUNK : (pst_idx + 1) * CHUNK
                                    ],
                                    src_ps[0:M2, :],
                                )

                    sig = sigp.tile([MP, GROUPS, CHUNK], f16, tag="sig")
                    sig_insts.append(
                        nc.scalar.activation(
                            sig[:, :, 0:cw],
                            logits[:, :, 0:cw],
                            mybir.ActivationFunctionType.Sigmoid,
                            bias=bias_s[:],
                        )
                    )

                    # group-sum: DVE pair-add tree for groups 0-3; group 4 is
                    # contracted by a second selector matmul instead of a
                    # third chained add (v1 showed 3 RAW-chained DVE ops
                    # inflate to ~1.2us/chunk; GpSimd in the chain stalled
                    # the PE sel slot 600-700ns/chunk via late ssum)
                    s2 = treep.tile([MP, 2, CHUNK], f16, tag="s2")
                    s4 = treep.tile([MP, CHUNK], f16, tag="s4")
                    nc.vector.tensor_add(
                        s2[:, :, 0:cw], sig[:, 0:2, 0:cw], sig[:, 2:4, 0:cw]
                    )
                    nc.vector.tensor_add(s4[:, 0:cw], s2[:, 0, 0:cw], s2[:, 1, 0:cw])

                    last = i == len(widths) - 1
                    pending_sels.append(
                        (
                            (tot_ps[:, 0:cw], sel_s[:, i, :], s4[:, 0:cw]),
                            dict(start=(i == 0), stop=False),
                            None,
                        )
                    )
                    pending_sels.append(
                        (
                            (tot_ps[:, 0:cw], sel_s[:, i, :], sig[:, 4, 0:cw]),
                            dict(start=False, stop=last),
                            (tot_ps, st) if last else None,
                        )
                    )
                    c0 += cw

            # flush remaining sel matmuls
            while pending_sels:
                a, k, pack = pending_sels.pop(0)
                nc.tensor.matmul(*a, **k)
                if pack is not None:
                    src_ps, pst_idx = pack
                    nc.vector.tensor_copy(
                        totals_sb[0:M2, pst_idx * CHUNK : (pst_idx + 1) * CHUNK],
                        src_ps[0:M2, :],
                    )

            # ---------------- epilogue: logit(total/S) = Ln(t) - Ln(S - t) ----
            # segmented in 4-supertile pieces so Ln / sub / DMA pipeline, with
            # output DMAs rotated across 3 engine queues (serial issue on one
            # sync queue costs ~0.6us per strided DMA). Unused regions of
            # ragged supertiles hold zeros -> -inf, never DMAed.
            wtot = n_st * CHUNK
            lnt = epi.tile([M2P, wtot], f32, tag="lnt")
            ln40 = epi.tile([M2P, wtot], f32, tag="ln40")
            outp = epi.tile([M2P, wtot], f32, tag="outp")
            dma_engines = [nc.sync, nc.gpsimd]
            # front-loaded segments; a short last segment minimizes the final
            # ln->sub->dma dependency tail
            if n_st >= 10:
                seg_bounds = sorted(set([0, 7, 12, 15, n_st - 1, n_st]))
            else:
                seg_bounds = [0, n_st]
            n_seg = len(seg_bounds) - 1
            dma_idx = 0
            for j in range(n_seg):
                lo, hi = seg_bounds[j], seg_bounds[j + 1]
                c_lo, c_hi = lo * CHUNK, hi * CHUNK
                ln_insts.append(
                    nc.scalar.activation(
                        lnt[0:M2, c_lo:c_hi],
                        totals_sb[0:M2, c_lo:c_hi],
                        mybir.ActivationFunctionType.Ln,
                        bias=czero[0:M2, :],
                    )
                )
                ln_insts.append(
                    nc.scalar.activation(
                        ln40[0:M2, c_lo:c_hi],
                        totals_sb[0:M2, c_lo:c_hi],
                        mybir.ActivationFunctionType.Ln,
                        bias=cS[0:M2, :],
                        scale=-1.0,
                    )
                )
                nc.vector.tensor_sub(
                    outp[0:M2, c_lo:c_hi], lnt[0:M2, c_lo:c_hi], ln40[0:M2, c_lo:c_hi]
                )
                # split each segment's output across both DMA queues by
                # partition halves: the final barrier otherwise waits ~4us on
                # the last big single-queue transfer
                nc.sync.dma_start(out_d[0:64, c_lo:c_hi], outp[0:64, c_lo:c_hi])
                nc.gpsimd.dma_start(
                    out_d[64:M2, c_lo:c_hi], outp[64:M2, c_lo:c_hi]
                )

            # keep every Ln after the last Sigmoid: exactly one ACT
            # table-set switch instead of one per supertile
            import os as _os

            if not _os.environ.get("KERNEL_NO_ACT_ORDER"):
                last_sig = sig_insts[-1]
                for li in ln_insts:
                    tile.add_dep_helper(
                        li.ins, last_sig.ins, sync=False, reason="act table order"
                    )
    nc.compile()
    return nc


def _host_inputs(features, weight, bias, masks, hw_total=HW):
    """Per-core input dicts. features [B,C,H,W] f32; weight [K,C]; bias [K];
    masks [S,B,C] int32."""
    fflat = np.asarray(features, np.float32).reshape(B, C, H * W)[:, :, :hw_total]
    w = np.asarray(weight, np.float32)
    bv = np.asarray(bias, np.float32)
    m = np.asarray(masks)

    # selector: sel[j, i, p] = 1 if j < M and p == 14*i + (j % 14)
    j = np.arange(M)
    sel = np.zeros((MP, SUPER, M2P), np.float16)
    for i in range(SUPER):
        sel[j, i, i * K + (j % K)] = 1.0

    biasv = np.zeros((MP, 1), np.float32)
    biasv[:M, 0] = bv[(j % K)]

    in_maps = []
    for b in range(B):
        fh = fflat[b].astype(np.float16)
        # weff[s, k, c] = w[k,c] * m[s, b, c] * 2
        weff = w[None, :, :] * (m[:, b, :].astype(np.float32) * 2.0)[:, None, :]
        # wall[c, g*MP + s_local*14 + k], rows M..MP-1 of each group zero
        wall = np.zeros((C, GROUPS, MP), np.float32)
        wall[:, :, :M] = (
            weff.reshape(GROUPS, SPG * K, C).transpose(2, 0, 1)
        )
        wall = wall.reshape(C, GROUPS * MP).astype(np.float16)
        in_maps.append({"fh": fh, "wall": wall, "sel": sel, "biasv": biasv})
    return in_maps


def _unpack_out(raw):
    """raw [M2P, n_st*CHUNK] packed as (chunk*K + k, st*CHUNK + col)."""
    sts = _chunk_layout(HW)
    raw = raw.reshape(M2P, len(sts) * CHUNK)
    out = np.empty((K, HW), np.float32)
    for st, (off, widths) in enumerate(sts):
        cc = 0
        for i, cw in enumerate(widths):
            out[:, off + cc : off + cc + cw] = raw[
                i * K : (i + 1) * K, st * CHUNK : st * CHUNK + cw
            ]
            cc += cw
    return out


def kernel(features, weight, bias, masks):
    from concourse.bass_utils import run_bass_kernel_spmd

    if "nc" not in _CACHE:
        _CACHE["nc"] = _build_program(HW)
    nc = _CACHE["nc"]

    in_maps = _host_inputs(features, weight, bias, masks)
    res = run_bass_kernel_spmd(nc, in_maps, core_ids=list(range(NCORES)))
    out = np.stack(
        [_unpack_out(r["out"]).reshape(K, H, W) for r in res.results], axis=0
    )
    return out.astype(np.float32)



# revision 35
# speedup vs baseline: 1.2492x; 1.2492x over previous
"""Trainium2 Bass kernel for nn_BayesianClassifier (MC-dropout 1x1-conv classifier).

Math: out = logit( mean_s sigmoid( (W * mask_s * 2) @ f + b ) )

Key restructuring vs the reference:
  - dropout2d masks fold into per-sample weight matrices on the host
    (einsum(f * m, W) == einsum(f, W * m)), so features are read ONCE.
  - Data-parallel over batch: core b handles features[b] ([256, 40000]).
  - Per core: 40 samples x 14 classes = 560 output rows, processed as
    5 groups of 112 live rows ((s_local, k) pairs, 8 samples/group),
    padded to 128 rows so every stationary operand has exactly 128
    columns (enables PE Fast Weight Load).
    Per hw-chunk of 256 columns:
      * 10 matmuls (5 groups x 2 c-halves) fp16 -> PSUM [128, 5, 256] (3 banks)
      * 1 merged sigmoid ACT (per-partition bias = bias[k]) -> SBUF fp16
      * DVE pair-add + add, GpSimd final add: sums the 5 groups -> [128, 256]
      * selector matmul (fp16 0/1 matrix) contracts the 8 samples and
        scatters the 14 classes of chunk i into rows 14i..14i+14 of a
        PSUM accumulator [128, 256] shared by 9 consecutive chunks.
  - Totals are packed into SBUF; the logit epilogue (Ln(t) - Ln(40-t))
    is forced after all sigmoids (explicit dep) so the ACT table set
    switches exactly once.
"""

import numpy as np

B, C, H, W = 8, 256, 200, 200
S, K = 40, 14
HW = H * W  # 40000
GROUPS = 5
SPG = 8  # samples per group
M = SPG * K  # 112 live rows per matmul group
MP = 128  # padded rows (FWL needs 128-column stationary operands)
CHUNK = 304  # 5*304*4B = 6080 <= 3 PSUM banks; fewer chunks -> fewer ACT
# instruction overheads (352 cyc each) on the bottleneck scalar queue
BANK_F32 = 512
SUPER = 9  # chunks per supertile
M2 = SUPER * K  # 126 live totals rows
M2P = 128
NCORES = 8

_CACHE = {}


def _chunk_layout(hw_total):
    """Returns list of supertiles; each is (hw_offset, [chunk_widths]).
    The first supertile is short (128+256 cols) so its feature DMA lands
    fast and the first sigmoid issues as early as possible."""
    sts = []
    off = 0
    first = True
    while off < hw_total:
        if first:
            widths = [128, CHUNK]
            first = False
        else:
            w_st = min(SUPER * CHUNK, hw_total - off)
            widths = []
            rem = w_st
            while rem > 0:
                cw = min(CHUNK, rem)
                widths.append(cw)
                rem -= cw
        sts.append((off, widths))
        off += sum(widths)
    return sts


def _bank_windows(g, cw):
    """Split group g's [g*CHUNK, g*CHUNK+cw) PSUM window at 512-f32 bank
    lines: a single matmul output must stay within one PSUM bank."""
    lo, hi = g * CHUNK, g * CHUNK + cw
    cuts = [lo]
    b = (lo // BANK_F32 + 1) * BANK_F32
    while b < hi:
        cuts.append(b)
        b += BANK_F32
    cuts.append(hi)
    return [(a - lo, b2 - a) for a, b2 in zip(cuts, cuts[1:])]


def _build_program(hw_total):
    import concourse.bass as bass
    import concourse.bacc as bacc
    import concourse.tile as tile
    import concourse.mybir as mybir

    dt = mybir.dt
    f16, f32 = dt.float16, dt.float32

    nc = bacc.Bacc("TRN2", target_bir_lowering=False, debug=False)

    fh_d = nc.dram_tensor("fh", [C, hw_total], f16, kind="ExternalInput")
    wall_d = nc.dram_tensor("wall", [C, GROUPS * MP], f16, kind="ExternalInput")
    sel_d = nc.dram_tensor("sel", [MP, SUPER, M2P], f16, kind="ExternalInput")
    bias_d = nc.dram_tensor("biasv", [MP, 1], f32, kind="ExternalInput")

    sts = _chunk_layout(hw_total)
    n_st = len(sts)
    # output stays in the packed (chunk*14+k, st*256+col) totals layout; the
    # host un-permutes. A [K, hw] scatter DMA costs 126 descriptors/supertile
    # (~1.1us queue issue each, ~21us total); the packed layout is one fat
    # contiguous DMA per epilogue segment.
    out_d = nc.dram_tensor("out", [M2P, n_st * CHUNK], f32, kind="ExternalOutput")

    sig_insts = []
    ln_insts = []

    with tile.TileContext(nc) as tc:
        with (
            tc.tile_pool(name="const", bufs=1) as constp,
            tc.tile_pool(name="fpool", bufs=2) as fpool,
            tc.tile_pool(name="sigp", bufs=7) as sigp,
            tc.tile_pool(name="treep", bufs=7) as treep,
            tc.tile_pool(name="totsb", bufs=1) as totsb,
            tc.tile_pool(name="epi", bufs=1) as epi,
            tc.tile_pool(name="psl", bufs=2, space=bass.MemorySpace.PSUM) as psl,
            tc.tile_pool(name="pst", bufs=2, space=bass.MemorySpace.PSUM) as pst,
        ):
            # first supertile's feature tiles get the first sync-queue slots
            # (everything downstream waits on them); consts follow
            off0, widths0 = sts[0]
            w_st0 = sum(widths0)
            f0_first = fpool.tile(
                [128, w_st0], f16, tag="f0", padded_shape=[128, SUPER * CHUNK]
            )
            f1_first = fpool.tile(
                [128, w_st0], f16, tag="f1", padded_shape=[128, SUPER * CHUNK]
            )
            wall0 = constp.tile([128, GROUPS * MP], f16)
            wall1 = constp.tile([128, GROUPS * MP], f16)
            sel_s = constp.tile([MP, SUPER, M2P], f16)
            bias_s = constp.tile([MP, 1], f32)
            czero = constp.tile([M2P, 1], f32)
            cS = constp.tile([M2P, 1], f32)
            warm = constp.tile([128, 512], f16)
            scratch = constp.tile([M2P, 1], f32)
            nc.gpsimd.memset(warm[:], 0.0)
            nc.gpsimd.memset(czero[:], 0.0)
            nc.gpsimd.memset(cS[:], float(S))
            # spread the startup DMAs across 3 DMA-capable queues: each
            # dma_start costs ~600ns of queue issue time, so serializing them
            # on one queue delays the first matmul by ~4us
            nc.sync.dma_start(f0_first[:], fh_d[0:128, off0 : off0 + w_st0])
            nc.gpsimd.dma_start(wall0[:], wall_d[0:128, :])
            nc.scalar.dma_start(f1_first[:], fh_d[128:256, off0 : off0 + w_st0])
            nc.sync.dma_start(wall1[:], wall_d[128:256, :])
            nc.gpsimd.dma_start(bias_s[:], bias_d[:])
            nc.sync.dma_start(sel_s[:], sel_d[:])
            # dummy first-in-program sigmoid with no DMA deps: hoists the
            # sigmoid ACT_TABLE_LOAD to ~7us (it otherwise sits behind the
            # first chunk's matmul-wait sems, costing 1.3us on the critical
            # path)
            nc.scalar.activation(
                scratch[:],
                czero[:],
                mybir.ActivationFunctionType.Sigmoid,
                bias=czero[:],
            )
            # PE warm-up: HAM un-throttles after ~3.4us of sustained matmul
            # activity; without this, chunks 0-2 run at the 1.2GHz cold rate
            # while the feature/weight DMAs land (~5us of ramp)
            wtile = psl.tile([MP, GROUPS, CHUNK], f32, tag="logits")
            for _ in range(12):
                nc.tensor.matmul(
                    wtile[:, 0, :],
                    warm[:, 0:128],
                    warm[:, 0:CHUNK],
                    start=True,
                    stop=True,
                )

            totals_sb = totsb.tile([M2P, n_st * CHUNK], f32)

            # ---------------- main loop ----------------
            # sel matmuls are emitted SEL_LAG chunks late (global queue,
            # crossing supertile boundaries) so the sigmoid->DVE->GpSimd
            # chain latency never stalls the PE FIFO.
            SEL_LAG = 4
            pending_sels = []
            for st, (off, widths) in enumerate(sts):
                w_st = sum(widths)
                if st == 0:
                    f0, f1 = f0_first, f1_first
                else:
                    f0 = fpool.tile(
                        [128, w_st], f16, tag="f0", padded_shape=[128, SUPER * CHUNK]
                    )
                    f1 = fpool.tile(
                        [128, w_st], f16, tag="f1", padded_shape=[128, SUPER * CHUNK]
                    )
                    nc.sync.dma_start(f0[:], fh_d[0:128, off : off + w_st])
                    nc.gpsimd.dma_start(f1[:], fh_d[128:256, off : off + w_st])

                tot_ps = pst.tile([M2P, CHUNK], f32, tag="totals")

                c0 = 0
                for i, cw in enumerate(widths):
                    logits = psl.tile([MP, GROUPS, CHUNK], f32, tag="logits")
                    for g in range(GROUPS):
                        for wo, ww in _bank_windows(g, cw):
                            nc.tensor.matmul(
                                logits[:, g, wo : wo + ww],
                                wall0[:, g * MP : (g + 1) * MP],
                                f0[:, c0 + wo : c0 + wo + ww],
                                start=True,
                                stop=False,
                            )
                            nc.tensor.matmul(
                                logits[:, g, wo : wo + ww],
                                wall1[:, g * MP : (g + 1) * MP],
                                f1[:, c0 + wo : c0 + wo + ww],
                                start=False,
                                stop=True,
                            )
                    while len(pending_sels) >= 2 * SEL_LAG:
                        for _ in range(2):
                            a, k, pack = pending_sels.pop(0)
                            nc.tensor.matmul(*a, **k)
                            if pack is not None:
                                src_ps, pst_idx = pack
                                nc.vector.tensor_copy(
                                    totals_sb[
                                        0:M2, pst_idx * CHUNK : (pst_idx + 1) * CHUNK
                                    ],
                                    src_ps[0:M2, :],
                                )

                    sig = sigp.tile([MP, GROUPS, CHUNK], f16, tag="sig")
                    sig_insts.append(
                        nc.scalar.activation(
                            sig[:, :, 0:cw],
                            logits[:, :, 0:cw],
                            mybir.ActivationFunctionType.Sigmoid,
                            bias=bias_s[:],
                        )
                    )

                    # group-sum: DVE pair-add tree for groups 0-3; group 4 is
                    # contracted by a second selector matmul instead of a
                    # third chained add (v1 showed 3 RAW-chained DVE ops
                    # inflate to ~1.2us/chunk; GpSimd in the chain stalled
                    # the PE sel slot 600-700ns/chunk via late ssum)
                    s2 = treep.tile([MP, 2, CHUNK], f16, tag="s2")
                    s4 = treep.tile([MP, CHUNK], f16, tag="s4")
                    nc.vector.tensor_add(
                        s2[:, :, 0:cw], sig[:, 0:2, 0:cw], sig[:, 2:4, 0:cw]
                    )
                    nc.vector.tensor_add(s4[:, 0:cw], s2[:, 0, 0:cw], s2[:, 1, 0:cw])

                    last = i == len(widths) - 1
                    pending_sels.append(
                        (
                            (tot_ps[:, 0:cw], sel_s[:, i, :], s4[:, 0:cw]),
                            dict(start=(i == 0), stop=False),
                            None,
                        )
                    )
                    pending_sels.append(
                        (
                            (tot_ps[:, 0:cw], sel_s[:, i, :], sig[:, 4, 0:cw]),
                            dict(start=False, stop=last),
                            (tot_ps, st) if last else None,
                        )
                    )
                    c0 += cw

            # flush remaining sel matmuls
            while pending_sels:
                a, k, pack = pending_sels.pop(0)
                nc.tensor.matmul(*a, **k)
                if pack is not None:
                    src_ps, pst_idx = pack
                    nc.vector.tensor_copy(
                        totals_sb[0:M2, pst_idx * CHUNK : (pst_idx + 1) * CHUNK],
                        src_ps[0:M2, :],
                    )

            # ---------------- epilogue: logit(total/S) = Ln(t) - Ln(S - t) ----
            # segmented in 4-supertile pieces so Ln / sub / DMA pipeline, with
            # output DMAs rotated across 3 engine queues (serial issue on one
            # sync queue costs ~0.6us per strided DMA). Unused regions of
            # ragged supertiles hold zeros -> -inf, never DMAed.
            wtot = n_st * CHUNK
            lnt = epi.tile([M2P, wtot], f32, tag="lnt")
            ln40 = epi.tile([M2P, wtot], f32, tag="ln40")
            outp = epi.tile([M2P, wtot], f32, tag="outp")
            dma_engines = [nc.sync, nc.gpsimd]
            # front-loaded segments; a short last segment minimizes the final
            # ln->sub->dma dependency tail
            if n_st >= 17:
                seg_bounds = [0, 6, 12, 16, n_st]
            elif n_st >= 10:
                seg_bounds = [0, 6, 11, 15, n_st]
            else:
                seg_bounds = [0, n_st]
            n_seg = len(seg_bounds) - 1
            dma_idx = 0
            for j in range(n_seg):
                lo, hi = seg_bounds[j], seg_bounds[j + 1]
                c_lo, c_hi = lo * CHUNK, hi * CHUNK
                ln_insts.append(
                    nc.scalar.activation(
                        lnt[0:M2, c_lo:c_hi],
                        totals_sb[0:M2, c_lo:c_hi],
                        mybir.ActivationFunctionType.Ln,
                        bias=czero[0:M2, :],
                    )
                )
                ln_insts.append(
                    nc.scalar.activation(
                        ln40[0:M2, c_lo:c_hi],
                        totals_sb[0:M2, c_lo:c_hi],
                        mybir.ActivationFunctionType.Ln,
                        bias=cS[0:M2, :],
                        scale=-1.0,
                    )
                )
                nc.vector.tensor_sub(
                    outp[0:M2, c_lo:c_hi], lnt[0:M2, c_lo:c_hi], ln40[0:M2, c_lo:c_hi]
                )
                eng = dma_engines[dma_idx % len(dma_engines)]
                dma_idx += 1
                eng.dma_start(
                    out_d[0:M2, c_lo:c_hi], outp[0:M2, c_lo:c_hi]
                )

            # keep every Ln after the last Sigmoid: exactly one ACT
            # table-set switch instead of one per supertile
            import os as _os

            if not _os.environ.get("KERNEL_NO_ACT_ORDER"):
                last_sig = sig_insts[-1]
                for li in ln_insts:
                    tile.add_dep_helper(
                        li.ins, last_sig.ins, sync=False, reason="act table order"
                    )
    nc.compile()
    return nc


def _host_inputs(features, weight, bias, masks, hw_total=HW):
    """Per-core input dicts. features [B,C,H,W] f32; weight [K,C]; bias [K];
    masks [S,B,C] int32."""
    fflat = np.asarray(features, np.float32).reshape(B, C, H * W)[:, :, :hw_total]
    w = np.asarray(weight, np.float32)
    bv = np.asarray(bias, np.float32)
    m = np.asarray(masks)

    # selector: sel[j, i, p] = 1 if j < M and p == 14*i + (j % 14)
    j = np.arange(M)
    sel = np.zeros((MP, SUPER, M2P), np.float16)
    for i in range(SUPER):
        sel[j, i, i * K + (j % K)] = 1.0

    biasv = np.zeros((MP, 1), np.float32)
    biasv[:M, 0] = bv[(j % K)]

    in_maps = []
    for b in range(B):
        fh = fflat[b].astype(np.float16)
        # weff[s, k, c] = w[k,c] * m[s, b, c] * 2
        weff = w[None, :, :] * (m[:, b, :].astype(np.float32) * 2.0)[:, None, :]
        # wall[c, g*MP + s_local*14 + k], rows M..MP-1 of each group zero
        wall = np.zeros((C, GROUPS, MP), np.float32)
        wall[:, :, :M] = (
            weff.reshape(GROUPS, SPG * K, C).transpose(2, 0, 1)
        )
        wall = wall.reshape(C, GROUPS * MP).astype(np.float16)
        in_maps.append({"fh": fh, "wall": wall, "sel": sel, "biasv": biasv})
    return in_maps


def _unpack_out(raw):
    """raw [M2P, n_st*CHUNK] packed as (chunk*K + k, st*CHUNK + col)."""
    sts = _chunk_layout(HW)
    raw = raw.reshape(M2P, len(sts) * CHUNK)
    out = np.empty((K, HW), np.float32)
    for st, (off, widths) in enumerate(sts):
        cc = 0
        for i, cw in enumerate(widths):
            out[:, off + cc : off + cc + cw] = raw[
                i * K : (i + 1) * K, st * CHUNK : st * CHUNK + cw
            ]
            cc += cw
    return out


def kernel(features, weight, bias, masks):
    from concourse.bass_utils import run_bass_kernel_spmd

    if "nc" not in _CACHE:
        _CACHE["nc"] = _build_program(HW)
    nc = _CACHE["nc"]

    in_maps = _host_inputs(features, weight, bias, masks)
    res = run_bass_kernel_spmd(nc, in_maps, core_ids=list(range(NCORES)))
    out = np.stack(
        [_unpack_out(r["out"]).reshape(K, H, W) for r in res.results], axis=0
    )
    return out.astype(np.float32)

